# revision 7
# baseline (speedup 1.0000x reference)
"""DiGCN inception-block (3 layers, 2 adjacencies) on 8 TRN2 NeuronCores.

Strategy (dst-sharded graph parallelism, v2):
  - Nodes are partitioned across the 8 cores (12544 rows each, node space
    padded to 100352). Each core owns the output rows for its node shard.
  - Per layer: x_{k+1} = x_k @ W0 + (A1 @ x_k) @ W1 + (A2 @ x_k) @ W2 + b
    (using A @ (x W) == (A x) W, so the sparse ops run on raw x).
  - Sparse op A @ x: edges are sorted by destination block (128 dst rows),
    grouped into CPR chunks of 128 edges per (block, src-range). Source rows
    are fetched with dma_gather (bf16, 256B rows) from the replicated x
    table in HBM. Gathers are BATCHED: one call per (block-group of G,
    range, adjacency) with G*CPR*128 indices, amortizing the ~1us SWDGE
    descriptor-generation cost on the Pool engine (the v1 bottleneck).
  - One-hot scatter matrix oh[e, d*CB+j] = attr[e,j] * (d == drel[e,j]) is
    built per dst block with two DVE tensor_tensor ops in the (d-major,
    j-minor) layout so every operand has unit inner stride -> DVE 2x mode.
    The iota operand (value d repeated CB times) is a host constant.
    TensorEngine accumulates psum[feat, dst] += G_chunk.T @ oh[:, j-slice]
    over the CB chunks of the block (PSUM accumulation = segment sum).
  - Loops are block-major: both adjacencies and the dense branch complete
    per block-group, so only small per-block [128,128] tiles persist.
  - dma_gather indices are int16, so the x table is addressed in 4 ranges
    of 25088 rows; each (block, range) group is padded to a uniform chunk
    count CPR so the SPMD program is identical on every core. Padded slots
    use index -1 (skipped by the DGE) and have attr == 0 in the one-hot.
  - Between layers the bf16 node shards are AllGathered into the next
    x table (internal Shared DRAM); layer 3 writes f32 shards directly.
"""

import sys

sys.path.insert(0, "/opt/trn_rl_repo")

import numpy as np
import ml_dtypes

from concourse import bass, mybir, bacc
import concourse.tile as tile
from concourse.bass_utils import run_bass_kernel_spmd

BF16 = ml_dtypes.bfloat16

NCORES = 8
F = 128  # feature dim (both in and out)
N = 100000  # real node count
NPAD = 100352  # 8 * 12544, multiple of 8*128
R = 4  # src ranges (int16 gather index limit)
G = 7  # dst blocks per gather group (98 = 14 * 7)
NIDX_MAX = 896  # max indices per dma_gather call (16KB single-packet SDMA cap: 1024*256B/16)
SINGLE_PACKET = True  # False lifts the 16KB cap (per-row packets) -> huge calls, less Pool time
PAD_IDX = 0  # index used for padded gather slots


def _prep_adjacency(src, dst, attr, NPAD):
    """Pack one adjacency into the uniform per-core grid."""
    NL = NPAD // NCORES
    B = NL // 128
    SR = NPAD // R
    per_core = []
    core = dst // NL
    for r in range(NCORES):
        m = core == r
        s = src[m].astype(np.int64)
        d = (dst[m] - r * NL).astype(np.int64)
        a = attr[m].astype(np.float32)
        b = d >> 7
        drel = (d & 127).astype(np.float32)
        q = s // SR
        srel = (s - q * SR).astype(np.int16)
        key = (b * R + q).astype(np.int64)
        order = np.argsort(key, kind="stable")
        key_s = key[order]
        counts = np.bincount(key_s, minlength=B * R)
        starts = np.concatenate([[0], np.cumsum(counts)[:-1]])
        pos = np.arange(len(key_s)) - starts[key_s]
        per_core.append((key_s, pos, srel[order], drel[order], a[order], counts))
    max_count = max(int(pc[5].max()) for pc in per_core) if len(src) else 0
    return per_core, max_count


def _finalize_adjacency(per_core, CPR, NPAD):
    """Build the per-core idx/drel/attr arrays for a given CPR.

    idx tokens are laid out [group g][range q][block in group][cap] so one
    dma_gather per (g, q) reads a contiguous token span. drel/attr stay in
    the [128, B*CB] layout with col = b*CB + q*CPR + s, row = edge slot p.
    """
    NL = NPAD // NCORES
    B = NL // 128
    CB = R * CPR
    cap = CPR * 128
    n_groups = (B + G - 1) // G
    idx_arrs, drel_arrs, attr_arrs = [], [], []
    for key_s, pos, srel, drel, a, counts in per_core:
        grid_src = np.full((B, R, cap), PAD_IDX, np.int16)
        grid_drel = np.zeros((B, R, cap), np.float32)
        grid_attr = np.zeros((B, R, cap), np.float32)
        bq_b = key_s // R
        bq_q = key_s % R
        grid_src[bq_b, bq_q, pos] = srel
        grid_drel[bq_b, bq_q, pos] = drel
        grid_attr[bq_b, bq_q, pos] = a
        # idx tokens: [g][q][b in g][cap]
        chunks = []
        for g in range(n_groups):
            blk = grid_src[g * G : (g + 1) * G]  # [G, R, cap]
            chunks.append(np.ascontiguousarray(blk.transpose(1, 0, 2)).reshape(-1))
        tokens = np.concatenate(chunks)  # [B*R*cap]
        wrapped = np.tile(tokens.reshape(-1, 16).T, (8, 1))  # [128, B*R*cap/16]
        idx_arrs.append(np.ascontiguousarray(wrapped))
        # drel/attr: [128, B*CB] with col = b*CB + q*CPR + s, row = p
        dr = grid_drel.reshape(B, R, CPR, 128).transpose(3, 0, 1, 2).reshape(128, B * CB)
        at = grid_attr.reshape(B, R, CPR, 128).transpose(3, 0, 1, 2).reshape(128, B * CB)
        drel_arrs.append(np.ascontiguousarray(dr).astype(BF16))
        attr_arrs.append(np.ascontiguousarray(at).astype(BF16))
    return idx_arrs, drel_arrs, attr_arrs


def _build_kernel(NPAD, CPR):
    NL = NPAD // NCORES
    B = NL // 128
    CB = R * CPR
    cap = CPR * 128
    n_groups = (B + G - 1) // G
    assert B % G == 0, (B, G)
    SR = NPAD // R
    IDXW = R * B * cap // 16  # idx free dim (int16 cols) per adjacency
    GIDX = R * G * cap // 16  # idx cols per (group, adjacency)
    NIDX = G * cap  # indices per gather call

    nc = bacc.Bacc("TRN2", target_bir_lowering=False, debug=False, num_devices=NCORES,
                   num_swdge_queues=4)
    dt = mybir.dt
    x_table = nc.declare_dram_parameter("input0", [NPAD, F], dt.bfloat16, isOutput=False)
    xT0_in = nc.declare_dram_parameter("input1", [128, NL], dt.bfloat16, isOutput=False)
    idx_in = [
        nc.declare_dram_parameter(f"input{2 + i}", [128, IDXW], dt.int16, isOutput=False)
        for i in range(2)
    ]
    drel_in = [
        nc.declare_dram_parameter(f"input{4 + i}", [128, B * CB], dt.bfloat16, isOutput=False)
        for i in range(2)
    ]
    attr_in = [
        nc.declare_dram_parameter(f"input{6 + i}", [128, B * CB], dt.bfloat16, isOutput=False)
        for i in range(2)
    ]
    w_in = nc.declare_dram_parameter("input8", [9 * 128, F], dt.bfloat16, isOutput=False)
    bias_in = nc.declare_dram_parameter("input9", [128, 3 * F], dt.float32, isOutput=False)
    iota_in = nc.declare_dram_parameter("input10", [128, 128 * CB], dt.bfloat16, isOutput=False)
    out_p = nc.declare_dram_parameter("output0", [NL, F], dt.float32, isOutput=True)

    table1 = nc.dram_tensor("table1", [NPAD, F], dt.bfloat16, addr_space="Shared")
    table2 = nc.dram_tensor("table2", [NPAD, F], dt.bfloat16, addr_space="Shared")
    shard = [nc.dram_tensor(f"shard{k}", [NL, F], dt.bfloat16) for k in range(2)]
    tables = [x_table, table1, table2]

    with tile.TileContext(nc) as tc:
        with (
            tc.tile_pool(name="persist", bufs=1) as pp,
            tc.tile_pool(name="g0", bufs=2) as gp0,
            tc.tile_pool(name="g1", bufs=2) as gp1,
            tc.tile_pool(name="g2", bufs=2) as gp2,
            tc.tile_pool(name="g3", bufs=2) as gp3,
            tc.tile_pool(name="idxp", bufs=4) as idxp,
            tc.tile_pool(name="ohp", bufs=4) as ohp,
            tc.tile_pool(name="ssp", bufs=20) as ssp,
            tc.tile_pool(name="outp", bufs=3) as outp,
            tc.tile_pool(name="psA", bufs=5, space="PSUM") as psA,
            tc.tile_pool(name="psB", bufs=2, space="PSUM") as psB,
        ):
            gpools = [gp0, gp1, gp2, gp3]
            # persistent tiles
            drel_t = [pp.tile([128, B * CB], dt.bfloat16, tag=f"drel{a}", name=f"drel{a}") for a in range(2)]
            attr_t = [pp.tile([128, B * CB], dt.bfloat16, tag=f"attr{a}", name=f"attr{a}") for a in range(2)]
            for a in range(2):
                nc.sync.dma_start(drel_t[a][:], drel_in[a][:])
                nc.sync.dma_start(attr_t[a][:], attr_in[a][:])
            iota_t = pp.tile([128, 128 * CB], dt.bfloat16, tag="iota")
            nc.sync.dma_start(iota_t[:], iota_in[:])
            w_t = pp.tile([128, 9, 128], dt.bfloat16, tag="w")
            nc.sync.dma_start(w_t[:], w_in[:].rearrange("(w i) o -> i w o", i=128))
            bias_t = pp.tile([128, 3 * F], dt.float32, tag="bias")
            nc.sync.dma_start(bias_t[:], bias_in[:])
            xT = pp.tile([128, NL], dt.bfloat16, tag="xT")
            nc.sync.dma_start(xT[:], xT0_in[:])

            for k in range(3):
                table = tables[k]
                if k > 0:
                    nc.sync.dma_start(xT[:], shard[k - 1][:], transpose=True)
                for g in range(n_groups):
                    b0 = g * G
                    # per-group index tiles (both adjacencies)
                    idx_t = []
                    for a in range(2):
                        it = idxp.tile([128, GIDX], dt.int16, tag="idx", name=f"idx{a}")
                        nc.sync.dma_start(it[:], idx_in[a][:, g * GIDX : (g + 1) * GIDX])
                        idx_t.append(it)
                    # gathers + scatter psums per adjacency
                    ss = [[None] * G, [None] * G]  # [a][i] -> sbuf [128,128] bf16
                    for a in range(2):
                        gts = []
                        for q in range(R):
                            gt = gpools[q].tile([128, G * CPR, F], dt.bfloat16, tag=f"g{q}", name=f"gt{q}a{a}")
                            for c0 in range(0, NIDX, NIDX_MAX):
                                ni = min(NIDX_MAX, NIDX - c0)
                                nc.gpsimd.dma_gather(
                                    out_ap=gt[:, c0 // 128 : (c0 + ni) // 128, :],
                                    in_ap=table[q * SR : (q + 1) * SR, :],
                                    idxs_ap=idx_t[a][
                                        :, (q * NIDX + c0) // 16 : (q * NIDX + c0 + ni) // 16
                                    ],
                                    num_idxs=ni,
                                    num_idxs_reg=ni,
                                    elem_size=F,
                                    queue_num=q,
                                )
                            gts.append(gt)
                        for i in range(G):
                            b = b0 + i
                            oh = ohp.tile([128, 128 * CB], dt.bfloat16, tag="oh", name="oh")
                            oh3 = oh[:].rearrange("p (d j) -> p d j", j=CB)
                            drel_b = (
                                drel_t[a][:, b * CB : (b + 1) * CB]
                                .unsqueeze(1)
                                .to_broadcast([128, 128, CB])
                            )
                            attr_b = (
                                attr_t[a][:, b * CB : (b + 1) * CB]
                                .unsqueeze(1)
                                .to_broadcast([128, 128, CB])
                            )
                            nc.vector.tensor_tensor(
                                out=oh3, in0=iota_t[:].rearrange("p (d j) -> p d j", j=CB),
                                in1=drel_b, op=mybir.AluOpType.is_equal
                            )
                            nc.vector.tensor_tensor(
                                out=oh3, in0=oh3, in1=attr_b, op=mybir.AluOpType.mult
                            )
                            ps = psA.tile([128, 128], dt.float32, tag="psA", name="psa")
                            for j in range(CB):
                                q, s = divmod(j, CPR)
                                nc.tensor.matmul(
                                    ps[:],
                                    gts[q][:, i * CPR + s, :],
                                    oh3[:, :, j],
                                    start=(j == 0),
                                    stop=(j == CB - 1),
                                )
                            st = ssp.tile([128, 128], dt.bfloat16, tag="ss", name=f"ss{a}")
                            nc.scalar.copy(st[:], ps[:])
                            ss[a][i] = st
                    # dense + bias + output per block
                    for i in range(G):
                        b = b0 + i
                        sl = slice(b * 128, (b + 1) * 128)
                        po = psB.tile([128, F], dt.float32, tag="psB", name="psb")
                        nc.tensor.matmul(po[:], ss[0][i][:], w_t[:, k * 3 + 1, :], start=True, stop=False)
                        nc.tensor.matmul(po[:], ss[1][i][:], w_t[:, k * 3 + 2, :], start=False, stop=False)
                        nc.tensor.matmul(po[:], xT[:, sl], w_t[:, k * 3 + 0, :], start=False, stop=True)
                        if k < 2:
                            ob = outp.tile([128, F], dt.bfloat16, tag="ob_bf", name="ob_bf")
                            nc.vector.tensor_tensor(
                                out=ob[:], in0=po[:], in1=bias_t[:, k * F : (k + 1) * F],
                                op=mybir.AluOpType.add,
                            )
                            nc.sync.dma_start(shard[k][sl, :], ob[:])
                        else:
                            ob = outp.tile([128, F], dt.float32, tag="ob_f32", name="ob_f32")
                            nc.vector.tensor_tensor(
                                out=ob[:], in0=po[:], in1=bias_t[:, k * F : (k + 1) * F],
                                op=mybir.AluOpType.add,
                            )
                            nc.sync.dma_start(out_p[sl, :], ob[:])
                if k < 2:
                    nc.gpsimd.collective_compute(
                        "AllGather",
                        mybir.AluOpType.bypass,
                        replica_groups=[list(range(NCORES))],
                        ins=[shard[k][:]],
                        outs=[tables[k + 1][:]],
                    )
    nc.finalize()
    return nc


def _run(x, edge_index, edge_attr, edge_index2, edge_attr2, weights, biases, NPAD,
         trace=False):
    """weights: [(W0,W1,W2)]*3 ; biases: [b_combined]*3 (already summed)."""
    NL = NPAD // NCORES
    B = NL // 128
    n = x.shape[0]

    adjs = []
    maxc = 0
    for (src, dst), attr in (
        (edge_index, edge_attr),
        (edge_index2, edge_attr2),
    ):
        pc, mc = _prep_adjacency(
            np.asarray(src, np.int64), np.asarray(dst, np.int64), attr, NPAD
        )
        adjs.append(pc)
        maxc = max(maxc, mc)
    CPR = max(1, -(-maxc // 128))
    CB = R * CPR
    data = [_finalize_adjacency(pc, CPR, NPAD) for pc in adjs]

    xpad = np.zeros((NPAD, x.shape[1]), np.float32)
    xpad[:n] = x
    xtab = xpad.astype(BF16)

    wstack = np.concatenate(
        [np.asarray(w, np.float32) for trio in weights for w in trio], axis=0
    ).astype(BF16)  # [9*128, 128]
    bstack = np.concatenate(
        [np.tile(np.asarray(b, np.float32)[None, :], (128, 1)) for b in biases], axis=1
    ).astype(np.float32)  # [128, 3*128]
    # iota_rep[p, d*CB + j] = d
    iota = np.repeat(np.arange(128, dtype=np.float32), CB)[None, :]
    iota = np.tile(iota, (128, 1)).astype(BF16)  # [128, 128*CB]

    in_maps = []
    for r in range(NCORES):
        xT0 = np.ascontiguousarray(xtab[r * NL : (r + 1) * NL].T)
        in_maps.append(
            {
                "input0": xtab,
                "input1": xT0,
                "input2": data[0][0][r],
                "input3": data[1][0][r],
                "input4": data[0][1][r],
                "input5": data[1][1][r],
                "input6": data[0][2][r],
                "input7": data[1][2][r],
                "input8": wstack,
                "input9": bstack,
                "input10": iota,
            }
        )

    nc = _build_kernel(NPAD, CPR)
    res = run_bass_kernel_spmd(nc, in_maps, list(range(NCORES)), trace=trace)
    out = np.concatenate([res.results[r]["output0"] for r in range(NCORES)], axis=0)
    return out[:n], res


def kernel(**inputs):
    x = np.asarray(inputs["x"], np.float32)
    weights = []
    biases = []
    for blk in ("b1", "b2", "b3"):
        weights.append(
            (
                np.asarray(inputs[f"{blk}_ln_w"], np.float32),
                np.asarray(inputs[f"{blk}_c1_w"], np.float32),
                np.asarray(inputs[f"{blk}_c2_w"], np.float32),
            )
        )
        biases.append(
            np.asarray(inputs[f"{blk}_ln_b"], np.float32)
            + np.asarray(inputs[f"{blk}_c1_b"], np.float32)
            + np.asarray(inputs[f"{blk}_c2_b"], np.float32)
        )
    out, _ = _run(
        x,
        np.asarray(inputs["edge_index"]),
        np.asarray(inputs["edge_attr"], np.float32),
        np.asarray(inputs["edge_index2"]),
        np.asarray(inputs["edge_attr2"], np.float32),
        weights,
        biases,
        NPAD,
    )
    return out
